# revision 8
# baseline (speedup 1.0000x reference)
"""Bag-of-words histogram kernel for Trainium2 (Bass/Tile), 8-core data-parallel.

Problem: docs [256, 2048] int32 token ids in [0, 32000) ->
         hist [256, 32000] fp32, hist[b, v] = count(docs[b, :] == v) / 2048.

Algorithm (per core, 32 rows): factor t = 256*hi + lo. For each 128-token
tile build oh_hi[t, hi] (DVE is_equal, value 2^-11 or 2^-12) and a lo-side
operand, then PSUM-accumulate hist[hi, lo] = oh_hi^T @ lo_operand on the PE
over 16 tiles per row.

Two row flavors balance DVE vs ACT (the two compare-capable engines):
  - plain rows: lo_operand = oh_lo (DVE is_equal, 256 cols). PSUM holds
    count*2^-11; copyback is one ACT copy.
  - sign rows: lo_operand = theta[t, j] = sign(lo_t + 0.5 - j) in {-1, +1}
    (ONE ACT Sign op, 258 cols, per-partition bias = lo+0.5). Then
    G[h, j] - G[h, j+1] = 2*count*2^-12 = count*2^-11, so the decode is a
    PSUM->SBUF copy plus a shifted tensor_tensor subtract on DVE.

Sharding: batch axis split 8 ways (32 rows per core), no communication.
"""

import sys

import numpy as np

for _p in ("/opt/trn_rl_repo",):
    if _p not in sys.path:
        sys.path.append(_p)

BATCH = 256
SEQ = 2048
VOCAB = 32000
N_CORES = 8
ROWS = BATCH // N_CORES  # 32 rows per core
P = 128
KT = SEQ // P            # 16 K-tiles per row
GR = 8                   # rows per input-DMA group
NLO = 256                # low-digit bins (t & 255)
NHI = 128                # high-digit compare width (t >> 8 < 125)
NTH = 258                # theta columns for sign rows (256 + guard + pad)

# Row flavors balance DVE vs ACT (the two compare-capable engines):
#   'plain'  - both one-hots on DVE, copyback on ACT.
#   'hisign' - hi side is an ACT Sign theta code theta[t,j]=sign(hi+0.5-j)
#              (ONE ACT op, 128 cols); G2[h]-G2[h+1] = count*2^-11 is
#              decoded with a partition-shifted SBUF->SBUF DMA + DVE
#              tensor_tensor subtract. lo one-hot (DVE) carries 2^-12.
#   'losign' - lo side is an ACT Sign theta code (258 cols); free-axis
#              shifted subtract on DVE decodes it.
def _flavor(r):
    if r % 3 != 1:
        return "hisign"
    return "plain"


FLAVOR = [_flavor(r) for r in range(ROWS)]


def _build_nc():
    from contextlib import ExitStack

    from concourse import bacc, bass, mybir
    from concourse.tile import TileContext

    nc = bacc.Bacc()
    docs = nc.dram_tensor("docs", [ROWS, SEQ], mybir.dt.int32, kind="ExternalInput")
    hist = nc.dram_tensor("hist", [ROWS, VOCAB], mybir.dt.float32, kind="ExternalOutput")

    f32 = mybir.dt.float32
    bf16 = mybir.dt.bfloat16
    Alu = mybir.AluOpType
    Act = mybir.ActivationFunctionType

    with TileContext(nc) as tc, ExitStack() as ctx:
        const_tp = ctx.enter_context(tc.tile_pool(name="const", bufs=1))
        tok_tp = ctx.enter_context(tc.tile_pool(name="tok", bufs=8))
        hilo_tp = ctx.enter_context(tc.tile_pool(name="hilo", bufs=8))
        ohhi_tp = ctx.enter_context(tc.tile_pool(name="ohhi", bufs=12))
        ohlo_tp = ctx.enter_context(tc.tile_pool(name="ohlo", bufs=10))
        th_tp = ctx.enter_context(tc.tile_pool(name="th", bufs=8))
        gs_tp = ctx.enter_context(tc.tile_pool(name="gs", bufs=4))
        res_tp = ctx.enter_context(tc.tile_pool(name="res", bufs=6))
        psum_tp = ctx.enter_context(tc.tile_pool(name="psum", bufs=7, space="PSUM"))

        # iota constants: value = column index, identical on every partition
        iota_hi = const_tp.tile([P, NHI], bf16)
        nc.gpsimd.iota(iota_hi[:], [[1, NHI]], channel_multiplier=0,
                       allow_small_or_imprecise_dtypes=True)
        iota_lo = const_tp.tile([P, NLO], bf16)
        nc.gpsimd.iota(iota_lo[:], [[1, NLO]], channel_multiplier=0,
                       allow_small_or_imprecise_dtypes=True)
        iota_th = const_tp.tile([P, NTH], bf16)
        nc.gpsimd.iota(iota_th[:], [[1, NTH]], channel_multiplier=0,
                       allow_small_or_imprecise_dtypes=True)

        for g in range(ROWS // GR):
            # Load GR rows; partition p holds tokens [16p, 16p+16) of each row
            # (any within-row permutation is histogram-invariant, so a fully
            # contiguous 64B-per-partition-line DMA is used).
            tok = tok_tp.tile([P, GR, KT], mybir.dt.int32)
            src = bass.AP(docs, g * GR * SEQ, [[16, P], [SEQ, GR], [1, KT]])
            nc.sync.dma_start(out=tok[:], in_=src)

            # hi = t >> 8, lo = t & 255; bit-vector ops cannot cast on HW,
            # so shift/and stay int32 and a mult-by-1.0 does the fp32 cast.
            hi_i = hilo_tp.tile([P, GR, KT], mybir.dt.int32, tag="hii")
            lo_i = hilo_tp.tile([P, GR, KT], mybir.dt.int32, tag="loi")
            nc.vector.tensor_scalar(out=hi_i[:], in0=tok[:], scalar1=8,
                                    scalar2=None, op0=Alu.logical_shift_right)
            nc.vector.tensor_scalar(out=lo_i[:], in0=tok[:], scalar1=255,
                                    scalar2=None, op0=Alu.bitwise_and)
            hi_pl = hilo_tp.tile([P, GR, KT], f32, tag="hi")
            lo_pl = hilo_tp.tile([P, GR, KT], f32, tag="lo")
            nc.vector.tensor_scalar(out=hi_pl[:], in0=hi_i[:], scalar1=1.0,
                                    scalar2=None, op0=Alu.mult)
            nc.vector.tensor_scalar(out=lo_pl[:], in0=lo_i[:], scalar1=1.0,
                                    scalar2=None, op0=Alu.mult)
            # theta biases for sign rows: lo + 0.5 / hi + 0.5
            lob = hilo_tp.tile([P, GR, KT], f32, tag="lob")
            nc.vector.tensor_scalar(out=lob[:], in0=lo_i[:], scalar1=1.0,
                                    scalar2=0.5, op0=Alu.mult, op1=Alu.add)
            hib = hilo_tp.tile([P, GR, KT], f32, tag="hib")
            nc.vector.tensor_scalar(out=hib[:], in0=hi_i[:], scalar1=1.0,
                                    scalar2=0.5, op0=Alu.mult, op1=Alu.add)

            for rl in range(GR):
                r = g * GR + rl
                flavor = FLAVOR[r]
                # one side of the product carries the 1/SEQ scale (exactly):
                # count*2^-11 lands in HBM directly for plain rows; sign rows
                # use 2^-12 so the +-theta difference restores 2^-11.
                hscale = {"plain": float(2.0 ** -11), "losign": float(2.0 ** -12),
                          "hisign": None}[flavor]
                ncols = NTH if flavor == "losign" else NLO
                ps_full = psum_tp.tile([P, NTH], f32)
                ps = ps_full[:, 0:ncols]
                for k in range(KT):
                    if flavor == "hisign":
                        lhs = th_tp.tile([P, NHI], bf16, tag="thh")
                        nc.scalar.activation(
                            out=lhs[:], in_=iota_hi[:], func=Act.Sign,
                            bias=hib[:, rl, k:k + 1], scale=-1.0)
                    else:
                        lhs = ohhi_tp.tile([P, NHI], bf16)
                        nc.vector.tensor_scalar(
                            out=lhs[:], in0=iota_hi[:],
                            scalar1=hi_pl[:, rl, k:k + 1], scalar2=hscale,
                            op0=Alu.is_equal, op1=Alu.mult)
                    if flavor == "losign":
                        rhs = th_tp.tile([P, NTH], bf16, tag="thl")
                        nc.scalar.activation(
                            out=rhs[:], in_=iota_th[:], func=Act.Sign,
                            bias=lob[:, rl, k:k + 1], scale=-1.0)
                    elif flavor == "hisign":
                        rhs = ohlo_tp.tile([P, NLO], bf16)
                        nc.vector.tensor_scalar(
                            out=rhs[:], in0=iota_lo[:],
                            scalar1=lo_pl[:, rl, k:k + 1],
                            scalar2=float(2.0 ** -12),
                            op0=Alu.is_equal, op1=Alu.mult)
                    else:
                        rhs = ohlo_tp.tile([P, NLO], bf16)
                        nc.vector.tensor_scalar(
                            out=rhs[:], in0=iota_lo[:],
                            scalar1=lo_pl[:, rl, k:k + 1], scalar2=None,
                            op0=Alu.is_equal)
                    nc.tensor.matmul(out=ps[:], lhsT=lhs[:], rhs=rhs[:],
                                     start=(k == 0), stop=(k == KT - 1))

                if flavor == "losign":
                    # hist[h, l] = G[h, l] - G[h, l+1]  (= count * 2^-11)
                    gs = gs_tp.tile([P, NTH], f32)
                    nc.vector.tensor_scalar(out=gs[:], in0=ps[:], scalar1=1.0,
                                            scalar2=None, op0=Alu.mult)
                    res = res_tp.tile([P, NLO], f32, tag="ress")
                    nc.vector.tensor_tensor(out=res[:], in0=gs[:, 0:NLO],
                                            in1=gs[:, 1:NLO + 1],
                                            op=Alu.subtract)
                elif flavor == "hisign":
                    # hist[h, l] = G2[h, l] - G2[h+1, l]: the partition shift
                    # is done by an SBUF->SBUF DMA (engines cannot cross
                    # partitions), then one DVE subtract.
                    gs = gs_tp.tile([P, NLO], f32, tag="gs2")
                    nc.vector.tensor_scalar(out=gs[:, :], in0=ps[:, :],
                                            scalar1=1.0, scalar2=None,
                                            op0=Alu.mult)
                    gsh = gs_tp.tile([P, NLO], f32, tag="gsh")
                    nc.sync.dma_start(out=gsh[0:126, :], in_=gs[1:127, :])
                    res = res_tp.tile([P, NLO], f32, tag="ress")
                    nc.vector.tensor_tensor(out=res[0:126, :], in0=gs[0:126, :],
                                            in1=gsh[0:126, :],
                                            op=Alu.subtract)
                else:
                    res = res_tp.tile([P, NLO], f32, tag="resp")
                    nc.scalar.copy(out=res[:], in_=ps[:])
                nc.sync.dma_start(
                    out=hist[r].rearrange("(h l) -> h l", l=NLO),
                    in_=res[:VOCAB // NLO, :])
    nc.compile()
    return nc


_NC_CACHE = None


def _get_nc():
    global _NC_CACHE
    if _NC_CACHE is None:
        _NC_CACHE = _build_nc()
    return _NC_CACHE


def run_sharded(docs: np.ndarray, trace: bool = False):
    """Run the 8-core SPMD kernel. Returns (full_output, BassKernelResults)."""
    from concourse.bass_utils import run_bass_kernel_spmd

    docs = np.ascontiguousarray(np.asarray(docs, dtype=np.int32))
    assert docs.shape == (BATCH, SEQ), docs.shape
    shards = docs.reshape(N_CORES, ROWS, SEQ)
    in_maps = [{"docs": shards[i]} for i in range(N_CORES)]
    res = run_bass_kernel_spmd(_get_nc(), in_maps, core_ids=list(range(N_CORES)),
                               trace=trace)
    out = np.concatenate([res.results[i]["hist"] for i in range(N_CORES)], axis=0)
    return out, res


def kernel(docs: np.ndarray) -> np.ndarray:
    out, _ = run_sharded(docs, trace=False)
    return out


# revision 12
# speedup vs baseline: 1.1550x; 1.1550x over previous
"""Bag-of-words histogram kernel for Trainium2 (Bass/Tile), 8-core data-parallel.

Problem: docs [256, 2048] int32 token ids in [0, 32000) ->
         hist [256, 32000] fp32, hist[b, v] = count(docs[b, :] == v) / 2048.

Algorithm (per core, 32 rows): factor t = 256*hi + lo. For each 128-token
tile build oh_hi[t, hi] (DVE is_equal, value 2^-11 or 2^-12) and a lo-side
operand, then PSUM-accumulate hist[hi, lo] = oh_hi^T @ lo_operand on the PE
over 16 tiles per row.

Two row flavors balance DVE vs ACT (the two compare-capable engines):
  - plain rows: lo_operand = oh_lo (DVE is_equal, 256 cols). PSUM holds
    count*2^-11; copyback is one ACT copy.
  - sign rows: lo_operand = theta[t, j] = sign(lo_t + 0.5 - j) in {-1, +1}
    (ONE ACT Sign op, 258 cols, per-partition bias = lo+0.5). Then
    G[h, j] - G[h, j+1] = 2*count*2^-12 = count*2^-11, so the decode is a
    PSUM->SBUF copy plus a shifted tensor_tensor subtract on DVE.

Sharding: batch axis split 8 ways (32 rows per core), no communication.
"""

import sys

import numpy as np

for _p in ("/opt/trn_rl_repo",):
    if _p not in sys.path:
        sys.path.append(_p)

BATCH = 256
SEQ = 2048
VOCAB = 32000
N_CORES = 8
ROWS = BATCH // N_CORES  # 32 rows per core
P = 128
KT = SEQ // P            # 16 K-tiles per row
GR = 8                   # rows per input-DMA group
NLO = 256                # low-digit bins (t & 255)
NHI = 128                # high-digit compare width (t >> 8 < 125)
NTH = 258                # theta columns for sign rows (256 + guard + pad)

# Row flavors balance DVE vs ACT (the two compare-capable engines):
#   'plain'  - both one-hots on DVE, copyback on ACT.
#   'hisign' - hi side is an ACT Sign theta code theta[t,j]=sign(hi+0.5-j)
#              (ONE ACT op, 128 cols); G2[h]-G2[h+1] = count*2^-11 is
#              decoded with a partition-shifted SBUF->SBUF DMA + DVE
#              tensor_tensor subtract. lo one-hot (DVE) carries 2^-12.
#   'losign' - lo side is an ACT Sign theta code (258 cols); free-axis
#              shifted subtract on DVE decodes it.
def _flavor(r):
    if r % 3 != 1:
        return "hisign"
    return "plain"


FLAVOR = [_flavor(r) for r in range(ROWS)]


def _build_nc():
    from contextlib import ExitStack

    from concourse import bacc, bass, mybir
    from concourse.tile import TileContext

    nc = bacc.Bacc()
    docs = nc.dram_tensor("docs", [ROWS, SEQ], mybir.dt.int32, kind="ExternalInput")
    hist = nc.dram_tensor("hist", [ROWS, VOCAB], mybir.dt.float32, kind="ExternalOutput")

    f32 = mybir.dt.float32
    bf16 = mybir.dt.bfloat16
    Alu = mybir.AluOpType
    Act = mybir.ActivationFunctionType

    with TileContext(nc) as tc, ExitStack() as ctx:
        const_tp = ctx.enter_context(tc.tile_pool(name="const", bufs=1))
        tok_tp = ctx.enter_context(tc.tile_pool(name="tok", bufs=8))
        hilo_tp = ctx.enter_context(tc.tile_pool(name="hilo", bufs=8))
        ohhi_tp = ctx.enter_context(tc.tile_pool(name="ohhi", bufs=12))
        ohlo_tp = ctx.enter_context(tc.tile_pool(name="ohlo", bufs=10))
        th_tp = ctx.enter_context(tc.tile_pool(name="th", bufs=8))
        gs_tp = ctx.enter_context(tc.tile_pool(name="gs", bufs=4))
        res_tp = ctx.enter_context(tc.tile_pool(name="res", bufs=6))
        psum_tp = ctx.enter_context(tc.tile_pool(name="psum", bufs=7, space="PSUM"))
        warm_tp = ctx.enter_context(tc.tile_pool(name="warm", bufs=1, space="PSUM"))

        # iota constants: value = column index, identical on every partition
        iota_hi = const_tp.tile([P, NHI], bf16)
        nc.gpsimd.iota(iota_hi[:], [[1, NHI]], channel_multiplier=0,
                       allow_small_or_imprecise_dtypes=True)
        iota_lo = const_tp.tile([P, NLO], bf16)
        nc.gpsimd.iota(iota_lo[:], [[1, NLO]], channel_multiplier=0,
                       allow_small_or_imprecise_dtypes=True)
        iota_th = const_tp.tile([P, NTH], bf16)
        nc.gpsimd.iota(iota_th[:], [[1, NTH]], channel_multiplier=0,
                       allow_small_or_imprecise_dtypes=True)
        # Keep-warm scratch: the PE HAM clock gate halves the PE clock after
        # any ~3.4us idle window; tiny dummy matmuls chained off each row's
        # one-hots keep activity in every window so real matmuls run at 2.4GHz.
        warm_ps = warm_tp.tile([P, 2], f32)

        for g in range(ROWS // GR):
            # Load GR rows; partition p holds tokens [16p, 16p+16) of each row
            # (any within-row permutation is histogram-invariant, so a fully
            # contiguous 64B-per-partition-line DMA is used).
            tok = tok_tp.tile([P, GR, KT], mybir.dt.int32)
            src = bass.AP(docs, g * GR * SEQ, [[16, P], [SEQ, GR], [1, KT]])
            nc.sync.dma_start(out=tok[:], in_=src)

            # hi = t >> 8, lo = t & 255; bit-vector ops cannot cast on HW,
            # so shift/and stay int32 and a mult-by-1.0 does the fp32 cast.
            hi_i = hilo_tp.tile([P, GR, KT], mybir.dt.int32, tag="hii")
            lo_i = hilo_tp.tile([P, GR, KT], mybir.dt.int32, tag="loi")
            nc.vector.tensor_scalar(out=hi_i[:], in0=tok[:], scalar1=8,
                                    scalar2=None, op0=Alu.logical_shift_right)
            nc.vector.tensor_scalar(out=lo_i[:], in0=tok[:], scalar1=255,
                                    scalar2=None, op0=Alu.bitwise_and)
            hi_pl = hilo_tp.tile([P, GR, KT], f32, tag="hi")
            lo_pl = hilo_tp.tile([P, GR, KT], f32, tag="lo")
            nc.vector.tensor_scalar(out=hi_pl[:], in0=hi_i[:], scalar1=1.0,
                                    scalar2=None, op0=Alu.mult)
            nc.vector.tensor_scalar(out=lo_pl[:], in0=lo_i[:], scalar1=1.0,
                                    scalar2=None, op0=Alu.mult)
            # theta biases for sign rows: lo + 0.5 / hi + 0.5
            lob = hilo_tp.tile([P, GR, KT], f32, tag="lob")
            nc.vector.tensor_scalar(out=lob[:], in0=lo_i[:], scalar1=1.0,
                                    scalar2=0.5, op0=Alu.mult, op1=Alu.add)
            hib = hilo_tp.tile([P, GR, KT], f32, tag="hib")
            nc.vector.tensor_scalar(out=hib[:], in0=hi_i[:], scalar1=1.0,
                                    scalar2=0.5, op0=Alu.mult, op1=Alu.add)

            for rl in range(GR):
                r = g * GR + rl
                flavor = FLAVOR[r]
                # one side of the product carries the 1/SEQ scale (exactly):
                # count*2^-11 lands in HBM directly for plain rows; sign rows
                # use 2^-12 so the +-theta difference restores 2^-11.
                hscale = {"plain": float(2.0 ** -11), "losign": float(2.0 ** -12),
                          "hisign": None}[flavor]
                ncols = NTH if flavor == "losign" else NLO
                ps_full = psum_tp.tile([P, NTH], f32)
                ps = ps_full[:, 0:ncols]
                for k in range(KT):
                    if flavor == "hisign":
                        lhs = th_tp.tile([P, NHI], bf16, tag="thh")
                        nc.scalar.activation(
                            out=lhs[:], in_=iota_hi[:], func=Act.Sign,
                            bias=hib[:, rl, k:k + 1], scale=-1.0)
                    else:
                        lhs = ohhi_tp.tile([P, NHI], bf16)
                        nc.vector.tensor_scalar(
                            out=lhs[:], in0=iota_hi[:],
                            scalar1=hi_pl[:, rl, k:k + 1], scalar2=hscale,
                            op0=Alu.is_equal, op1=Alu.mult)
                    if flavor == "losign":
                        rhs = th_tp.tile([P, NTH], bf16, tag="thl")
                        nc.scalar.activation(
                            out=rhs[:], in_=iota_th[:], func=Act.Sign,
                            bias=lob[:, rl, k:k + 1], scale=-1.0)
                    elif flavor == "hisign":
                        rhs = ohlo_tp.tile([P, NLO], bf16)
                        nc.vector.tensor_scalar(
                            out=rhs[:], in0=iota_lo[:],
                            scalar1=lo_pl[:, rl, k:k + 1],
                            scalar2=float(2.0 ** -12),
                            op0=Alu.is_equal, op1=Alu.mult)
                    else:
                        rhs = ohlo_tp.tile([P, NLO], bf16)
                        nc.vector.tensor_scalar(
                            out=rhs[:], in0=iota_lo[:],
                            scalar1=lo_pl[:, rl, k:k + 1], scalar2=None,
                            op0=Alu.is_equal)
                    nc.tensor.matmul(out=ps[:], lhsT=lhs[:], rhs=rhs[:],
                                     start=(k == 0), stop=(k == KT - 1))
                    if k in (5, 13):
                        nc.tensor.matmul(out=warm_ps[0:2, :], lhsT=rhs[:, 0:2],
                                         rhs=rhs[:, 0:2], start=True, stop=True)

                if flavor == "losign":
                    # hist[h, l] = G[h, l] - G[h, l+1]  (= count * 2^-11)
                    gs = gs_tp.tile([P, NTH], f32)
                    nc.vector.tensor_scalar(out=gs[:], in0=ps[:], scalar1=1.0,
                                            scalar2=None, op0=Alu.mult)
                    res = res_tp.tile([P, NLO], f32, tag="ress")
                    nc.vector.tensor_tensor(out=res[:], in0=gs[:, 0:NLO],
                                            in1=gs[:, 1:NLO + 1],
                                            op=Alu.subtract)
                elif flavor == "hisign":
                    # hist[h, l] = G2[h, l] - G2[h+1, l]: the partition shift
                    # is done by an SBUF->SBUF DMA (engines cannot cross
                    # partitions), then one DVE subtract.
                    gs = gs_tp.tile([P, NLO], f32, tag="gs2")
                    nc.vector.tensor_scalar(out=gs[:, :], in0=ps[:, :],
                                            scalar1=1.0, scalar2=None,
                                            op0=Alu.mult)
                    gsh = gs_tp.tile([P, NLO], f32, tag="gsh")
                    nc.sync.dma_start(out=gsh[0:126, :], in_=gs[1:127, :])
                    res = res_tp.tile([P, NLO], f32, tag="ress")
                    nc.vector.tensor_tensor(out=res[0:126, :], in0=gs[0:126, :],
                                            in1=gsh[0:126, :],
                                            op=Alu.subtract)
                else:
                    res = res_tp.tile([P, NLO], f32, tag="resp")
                    nc.scalar.copy(out=res[:], in_=ps[:])
                nc.sync.dma_start(
                    out=hist[r].rearrange("(h l) -> h l", l=NLO),
                    in_=res[:VOCAB // NLO, :])
    nc.compile()
    return nc


_NC_CACHE = None


def _get_nc():
    global _NC_CACHE
    if _NC_CACHE is None:
        _NC_CACHE = _build_nc()
    return _NC_CACHE


def run_sharded(docs: np.ndarray, trace: bool = False):
    """Run the 8-core SPMD kernel. Returns (full_output, BassKernelResults)."""
    from concourse.bass_utils import run_bass_kernel_spmd

    docs = np.ascontiguousarray(np.asarray(docs, dtype=np.int32))
    assert docs.shape == (BATCH, SEQ), docs.shape
    shards = docs.reshape(N_CORES, ROWS, SEQ)
    in_maps = [{"docs": shards[i]} for i in range(N_CORES)]
    res = run_bass_kernel_spmd(_get_nc(), in_maps, core_ids=list(range(N_CORES)),
                               trace=trace)
    out = np.concatenate([res.results[i]["hist"] for i in range(N_CORES)], axis=0)
    return out, res


def kernel(docs: np.ndarray) -> np.ndarray:
    out, _ = run_sharded(docs, trace=False)
    return out
